# revision 22
# baseline (speedup 1.0000x reference)
"""Distributed Trainium2 Bass kernel for nn_Attention_13125420057022.

Multi-head attention (B=2, S=2048, H=768, 12 heads, head_dim=64) with
interleaved RoPE, run SPMD on 8 NeuronCores.

Sharding: core c handles batch b=c//4 and query rows [512*(c%4), 512*(c%4+1)).
Data-parallel with replicated K/V: every core computes K^T and V for the FULL
sequence of its batch (the 4x redundant ~40us of warm TensorE time is cheaper
than a 4-rank ring AllGather, which measures ~90us serialized plus contended
read-back), plus Q for its own 512 rows. Attention and the output projection
then produce a disjoint slice of the output rows — no collectives at all.

Compute is bf16 with f32 PSUM accumulation. Scores are computed transposed
(S^T[k,q] = sum_d K^T[d,k] Q^T[d,q]) so the exp() output feeds the A.V matmul
directly with no on-chip transposes; two key blocks share one [128,1024] PSUM
tile so each ACT exp covers 1024 columns and TensorE runs long dependency-free
matmul waves (keeps the HAM clock at 2.4GHz). Softmax row-sums come from a
ones column appended to V; no max-subtraction is needed because the logits are
bounded. The per-query 1/sum broadcast runs on GpSimd (partition_broadcast);
PSUM evictions run on DVE so ACT does nothing but exp. RoPE uses a host-side
de-interleave permutation of the Q/K weight rows to turn interleaved rotation
into rotate-half form (contiguous 32-row block swaps via SB->SB DMA).
"""

import math
import sys
from contextlib import ExitStack

import numpy as np
import ml_dtypes

sys.path.insert(0, "/opt/trn_rl_repo")

import concourse.bass as bass  # noqa: E402
import concourse.mybir as mybir  # noqa: E402
import concourse.tile as tile  # noqa: E402
from concourse import bacc  # noqa: E402
from concourse.bass_utils import run_bass_kernel_spmd  # noqa: E402

BF16 = ml_dtypes.bfloat16
F32 = mybir.dt.float32
BF = mybir.dt.bfloat16

B, S, H = 2, 2048, 768
NH, HD = 12, 64
THETA = 10000.0
NCORES = 8
GROUP = 4  # cores per batch
SLOC = S // GROUP  # 512 query rows per core
NKB = S // 128  # 16 key blocks
NSC = S // 512  # 4 sequence chunks of 512

EXP = mybir.ActivationFunctionType.Exp


def build_graph():
    nc = bacc.Bacc(
        "TRN2",
        target_bir_lowering=False,
        debug=False,
        num_devices=NCORES,
    )

    # External inputs (per-core shards, host-prepped)
    xT = nc.dram_tensor("xT", [H, S], BF, kind="ExternalInput")  # full batch
    xTq = nc.dram_tensor("xTq", [H, SLOC], BF, kind="ExternalInput")  # own rows
    wt = nc.dram_tensor("wt", [H, 3 * H], BF, kind="ExternalInput")
    qkvb_qk = nc.dram_tensor("qkvb_qk", [128, 12], F32, kind="ExternalInput")
    qkvb_v = nc.dram_tensor("qkvb_v", [1, H], BF, kind="ExternalInput")
    projt = nc.dram_tensor("projt", [H, H], BF, kind="ExternalInput")
    projb = nc.dram_tensor("projb", [1, H], BF, kind="ExternalInput")
    cq = nc.dram_tensor("cq", [128, SLOC], BF, kind="ExternalInput")
    sq = nc.dram_tensor("sq", [128, SLOC], BF, kind="ExternalInput")
    ck = nc.dram_tensor("ck", [128, S], BF, kind="ExternalInput")
    sk = nc.dram_tensor("sk", [128, S], BF, kind="ExternalInput")
    out_ext = nc.dram_tensor("out", [SLOC, H], F32, kind="ExternalOutput")

    with tile.TileContext(nc) as tc, ExitStack() as ctx:
        singles = ctx.enter_context(tc.tile_pool(name="singles", bufs=1))
        kraw_p = ctx.enter_context(tc.tile_pool(name="kraw", bufs=2))
        kswp_p = ctx.enter_context(tc.tile_pool(name="kswp", bufs=1))
        ktmp_p = ctx.enter_context(tc.tile_pool(name="ktmp", bufs=1))
        qraw_p = ctx.enter_context(tc.tile_pool(name="qraw", bufs=2))
        qswp_p = ctx.enter_context(tc.tile_pool(name="qswp", bufs=2))
        qtmp_p = ctx.enter_context(tc.tile_pool(name="qtmp", bufs=1))
        v_pool = ctx.enter_context(tc.tile_pool(name="v_pool", bufs=1))
        at_pool = ctx.enter_context(tc.tile_pool(name="at", bufs=7))
        small_p = ctx.enter_context(tc.tile_pool(name="small", bufs=2))
        ctxn_p = ctx.enter_context(tc.tile_pool(name="ctxn", bufs=2))
        out_p = ctx.enter_context(tc.tile_pool(name="outp", bufs=2))

        # ---- SBUF tiles ----
        wt_sb = singles.tile([128, 6, 3 * H], BF)
        xT_sb = singles.tile([128, 6, S], BF)
        xTq_sb = singles.tile([128, 6, SLOC], BF)
        projt_sb = singles.tile([128, 6, H], BF)
        projb_sb = singles.tile([1, H], BF)
        qkvb_sb = singles.tile([128, 12], F32)
        qkvbv_sb = singles.tile([1, H], BF)
        cq_sb = singles.tile([128, SLOC], BF)
        sq_sb = singles.tile([128, SLOC], BF)
        ck_sb = singles.tile([128, S], BF)
        sk_sb = singles.tile([128, S], BF)
        ones_bf = singles.tile([1, 128], BF)
        qT_sb = singles.tile([128, 6, SLOC], BF)
        kT_sb = singles.tile([128, 6, S], BF)
        ctxT_sb = singles.tile([128, 6, SLOC], BF)

        wt_r = wt.ap().rearrange("(c p) n -> c p n", p=128)
        xT_r = xT.ap().rearrange("(c p) s -> c p s", p=128)
        xTq_r = xTq.ap().rearrange("(c p) s -> c p s", p=128)
        projt_r = projt.ap().rearrange("(c p) n -> c p n", p=128)
        # sync queue: x + K-columns of W interleaved so K block 0 starts ASAP
        nc.sync.dma_start(out=qkvb_sb[:], in_=qkvb_qk.ap())
        for c in range(6):
            nc.sync.dma_start(out=xT_sb[:, c, 0:1024], in_=xT_r[c][:, 0:1024])
            nc.sync.dma_start(out=wt_sb[:, c, 768:1536], in_=wt_r[c][:, 768:1536])
            nc.sync.dma_start(out=xT_sb[:, c, 1024:2048], in_=xT_r[c][:, 1024:2048])
        # gpsimd (SWDGE) queue: V/Q weight columns + proj weights + own-x
        for c in range(6):
            nc.gpsimd.dma_start(out=wt_sb[:, c, 1536:2304], in_=wt_r[c][:, 1536:2304])
        for c in range(6):
            nc.gpsimd.dma_start(out=xTq_sb[:, c, :], in_=xTq_r[c])
        for c in range(6):
            nc.gpsimd.dma_start(out=wt_sb[:, c, 0:768], in_=wt_r[c][:, 0:768])
        for c in range(6):
            nc.gpsimd.dma_start(out=projt_sb[:, c, :], in_=projt_r[c])
        nc.gpsimd.dma_start(out=projb_sb[:], in_=projb.ap())
        # scalar queue: rope tables + v-bias
        nc.scalar.dma_start(out=ck_sb[:], in_=ck.ap())
        nc.scalar.dma_start(out=sk_sb[:], in_=sk.ap())
        nc.scalar.dma_start(out=cq_sb[:], in_=cq.ap())
        nc.scalar.dma_start(out=sq_sb[:], in_=sq.ap())
        nc.scalar.dma_start(out=qkvbv_sb[:], in_=qkvb_v.ap())
        nc.vector.memset(ones_bf[:], 1.0)

        def rope(dst, raw, swp, cos_sb, sin_sb, tmp_pool, width):
            """dst = raw*cos + swp*sin (rotate-half form, swp pre-swapped)."""
            t1 = tmp_pool.tile([128, width], BF, tag="t1")
            t2 = tmp_pool.tile([128, width], BF, tag="t2")
            nc.vector.tensor_mul(t1[:], raw[:], cos_sb[:])
            nc.vector.tensor_mul(t2[:], swp[:], sin_sb[:])
            nc.vector.tensor_add(dst, t1[:], t2[:])

        def swap_dmas(swp, raw, width):
            nc.scalar.dma_start(out=swp[0:32, 0:width], in_=raw[32:64, 0:width])
            nc.scalar.dma_start(out=swp[32:64, 0:width], in_=raw[0:32, 0:width])
            nc.scalar.dma_start(out=swp[64:96, 0:width], in_=raw[96:128, 0:width])
            nc.scalar.dma_start(out=swp[96:128, 0:width], in_=raw[64:96, 0:width])

        with tc.tile_pool(name="st_ps", bufs=3, space="PSUM") as st_psum:
            # ---- K^T for the full sequence (heads 2t, 2t+1 per block) ----
            for t in range(6):
                nb = t + 6  # K columns of wt
                kraw = kraw_p.tile([128, S], BF)
                for sp in range(2):  # two [128,1024] psum pairs -> 2048 cols
                    ps = st_psum.tile([128, 1024], F32, tag="st")
                    for half in range(2):
                        sc = 2 * sp + half
                        for c in range(6):
                            nc.tensor.matmul(
                                ps[:, half * 512 : (half + 1) * 512],
                                lhsT=wt_sb[:, c, nb * 128 : (nb + 1) * 128],
                                rhs=xT_sb[:, c, sc * 512 : (sc + 1) * 512],
                                start=(c == 0),
                                stop=(c == 5),
                            )
                    nc.vector.tensor_scalar_add(
                        kraw[:, sp * 1024 : (sp + 1) * 1024],
                        ps[:],
                        qkvb_sb[:, nb : nb + 1],
                    )
                kswp = kswp_p.tile([128, S], BF)
                swap_dmas(kswp, kraw, S)
                rope(kT_sb[:, t, :], kraw, kswp, ck_sb, sk_sb, ktmp_p, S)

            # ---- V for the full sequence (natural layout, ones column) ----
            v_tiles = []
            with tc.tile_pool(name="v_ps", bufs=1, space="PSUM") as v_psum:
                for i in range(NKB):
                    vps = v_psum.tile([128, H], F32)
                    for c in range(6):
                        lhsT = xT_sb[:, c, i * 128 : (i + 1) * 128]
                        nc.tensor.matmul(
                            vps[:, 0:512],
                            lhsT=lhsT,
                            rhs=wt_sb[:, c, 1536:2048],
                            start=(c == 0),
                            stop=False,
                        )
                        nc.tensor.matmul(
                            vps[:, 512:768],
                            lhsT=lhsT,
                            rhs=wt_sb[:, c, 2048:2304],
                            start=(c == 0),
                            stop=False,
                        )
                    # bias via ones-row (rank-1 update), also closes the groups
                    nc.tensor.matmul(
                        vps[:, 0:512],
                        lhsT=ones_bf[:, 0:128],
                        rhs=qkvbv_sb[:, 0:512],
                        start=False,
                        stop=True,
                    )
                    nc.tensor.matmul(
                        vps[:, 512:768],
                        lhsT=ones_bf[:, 0:128],
                        rhs=qkvbv_sb[:, 512:768],
                        start=False,
                        stop=True,
                    )
                    vt = v_pool.tile([128, NH * 65], BF, tag=f"v{i}")
                    vt3 = vt.rearrange("p (h c) -> p h c", h=NH)
                    nc.vector.tensor_copy(
                        vt3[:, :, 0:64], vps.rearrange("p (h d) -> p h d", h=NH)
                    )
                    nc.vector.memset(vt3[:, :, 64:65], 1.0)
                    v_tiles.append(vt)

            # ---- Q^T for own rows ----
            for t in range(6):
                ps = st_psum.tile([128, 1024], F32, tag="st")
                for c in range(6):
                    nc.tensor.matmul(
                        ps[:, 0:512],
                        lhsT=wt_sb[:, c, t * 128 : (t + 1) * 128],
                        rhs=xTq_sb[:, c, :],
                        start=(c == 0),
                        stop=(c == 5),
                    )
                qraw = qraw_p.tile([128, SLOC], BF)
                nc.vector.tensor_scalar_add(
                    qraw[:], ps[:, 0:512], qkvb_sb[:, t : t + 1]
                )
                qswp = qswp_p.tile([128, SLOC], BF)
                swap_dmas(qswp, qraw, SLOC)
                rope(qT_sb[:, t, :], qraw, qswp, cq_sb, sq_sb, qtmp_p, SLOC)

            # ---- attention: per head, paired S^T -> exp -> A^T.V ----
            with tc.tile_pool(name="ctx_ps", bufs=2, space="PSUM") as ctx_psum:

                def head_mm(h):
                    t, r0 = h // 2, (h % 2) * 64
                    ctxp = ctx_psum.tile([65, SLOC], F32, tag="ctx")
                    for kp in range(NKB // 2):
                        st = st_psum.tile([128, 1024], F32, tag="st")
                        at = at_pool.tile([128, 1024], BF)
                        for half in range(2):
                            kb = 2 * kp + half
                            nc.tensor.matmul(
                                st[:, half * 512 : (half + 1) * 512],
                                lhsT=kT_sb[r0 : r0 + 64, t, kb * 128 : (kb + 1) * 128],
                                rhs=qT_sb[r0 : r0 + 64, t, :],
                                start=True,
                                stop=True,
                            )
                        nc.scalar.activation(out=at[:], in_=st[:], func=EXP)
                        for half in range(2):
                            kb = 2 * kp + half
                            nc.tensor.matmul(
                                ctxp[:],
                                lhsT=v_tiles[kb][:, h * 65 : (h + 1) * 65],
                                rhs=at[:, half * 512 : (half + 1) * 512],
                                start=(kb == 0),
                                stop=(kb == NKB - 1),
                            )
                    return ctxp

                def head_epilogue(h, ctxp):
                    t, r0 = h // 2, (h % 2) * 64
                    sums64 = small_p.tile([128, SLOC], F32, tag="sums64")
                    nc.vector.tensor_copy(sums64[64:65, :], ctxp[64:65, :])
                    sums = small_p.tile([1, SLOC], F32, tag="sums")
                    nc.scalar.dma_start(out=sums[:], in_=sums64[64:65, :])
                    rec = small_p.tile([1, SLOC], F32, tag="rec")
                    nc.vector.reciprocal_approx_fast(out=rec[:], in_=sums[:])
                    bc_sb = small_p.tile([64, SLOC], F32, tag="bc_sb")
                    nc.gpsimd.partition_broadcast(bc_sb[:], rec[:], channels=64)
                    if r0 == 0:
                        nc.vector.tensor_mul(
                            ctxT_sb[0:64, t, :], ctxp[0:64, :], bc_sb[:]
                        )
                    else:
                        cn = ctxn_p.tile([64, SLOC], BF)
                        nc.vector.tensor_mul(cn[:], ctxp[0:64, :], bc_sb[:])
                        nc.scalar.dma_start(out=ctxT_sb[64:128, t, :], in_=cn[:])

                prev = None
                for h in range(NH):
                    ctxp = head_mm(h)
                    if prev is not None:
                        head_epilogue(h - 1, prev)
                    prev = ctxp
                head_epilogue(NH - 1, prev)

        # ---- output projection: out[s,:] = ctx^T.T @ projt + projb ----
        with tc.tile_pool(name="o_ps", bufs=2, space="PSUM") as o_psum:
            for i in range(4):
                ops = o_psum.tile([128, H], F32)
                for c in range(6):
                    lhsT = ctxT_sb[:, c, i * 128 : (i + 1) * 128]
                    nc.tensor.matmul(
                        ops[:, 0:512],
                        lhsT=lhsT,
                        rhs=projt_sb[:, c, 0:512],
                        start=(c == 0),
                        stop=False,
                    )
                    nc.tensor.matmul(
                        ops[:, 512:768],
                        lhsT=lhsT,
                        rhs=projt_sb[:, c, 512:768],
                        start=(c == 0),
                        stop=False,
                    )
                nc.tensor.matmul(
                    ops[:, 0:512],
                    lhsT=ones_bf[:, 0:128],
                    rhs=projb_sb[:, 0:512],
                    start=False,
                    stop=True,
                )
                nc.tensor.matmul(
                    ops[:, 512:768],
                    lhsT=ones_bf[:, 0:128],
                    rhs=projb_sb[:, 512:768],
                    start=False,
                    stop=True,
                )
                osb = out_p.tile([128, H], F32)
                nc.vector.tensor_copy(osb[:], ops[:])
                nc.sync.dma_start(
                    out=out_ext.ap()[i * 128 : (i + 1) * 128, :], in_=osb[:]
                )

    nc.compile()
    return nc


_PERM = np.concatenate([np.arange(0, HD, 2), np.arange(1, HD, 2)])


def prep_inputs(x, qkv_w, qkv_b, proj_w, proj_b):
    """Shard + lay out the full inputs into per-core input maps."""
    x = np.asarray(x, np.float32)
    qkv_w = np.asarray(qkv_w, np.float32)
    qkv_b = np.asarray(qkv_b, np.float32)
    proj_w = np.asarray(proj_w, np.float32)
    proj_b = np.asarray(proj_b, np.float32)

    # de-interleave permutation of q/k head dims (rows of qkv_w)
    Wp = qkv_w.copy()
    bp = qkv_b.copy()
    for sec in range(2):
        for h in range(NH):
            base = sec * H + h * HD
            Wp[base : base + HD] = qkv_w[base + _PERM]
            bp[base : base + HD] = qkv_b[base + _PERM]
    wt = np.ascontiguousarray(Wp.T).astype(BF16)  # [768, 2304]
    qkvb_qk = np.ascontiguousarray(bp[: 2 * H].reshape(12, 128).T).astype(np.float32)
    qkvb_v = qkv_b[2 * H :].reshape(1, H).astype(BF16)
    projt = np.ascontiguousarray(proj_w.T).astype(BF16)
    projb = proj_b.reshape(1, H).astype(BF16)

    inv_freq = 1.0 / (THETA ** (np.arange(0, HD, 2, dtype=np.float32) / HD))
    angles = np.arange(S, dtype=np.float32)[None, :] * inv_freq[:, None]  # [32, S]
    cos_t, sin_t = np.cos(angles), np.sin(angles)
    qscale = 1.0 / math.sqrt(HD)

    ck_a = np.tile(cos_t, (4, 1)).astype(BF16)  # [128, S]
    sk_a = np.concatenate([-sin_t, sin_t, -sin_t, sin_t], axis=0).astype(BF16)
    xT_full = [np.ascontiguousarray(x[b].T).astype(BF16) for b in range(B)]

    in_maps = []
    for c in range(NCORES):
        b, j = c // GROUP, c % GROUP
        sl = slice(j * SLOC, (j + 1) * SLOC)
        cos_j, sin_j = cos_t[:, sl] * qscale, sin_t[:, sl] * qscale
        cq_a = np.tile(cos_j, (4, 1)).astype(BF16)
        sq_a = np.concatenate([-sin_j, sin_j, -sin_j, sin_j], axis=0).astype(BF16)
        in_maps.append(
            {
                "xT": xT_full[b],
                "xTq": np.ascontiguousarray(xT_full[b][:, sl]),
                "wt": wt,
                "qkvb_qk": qkvb_qk,
                "qkvb_v": qkvb_v,
                "projt": projt,
                "projb": projb,
                "cq": cq_a,
                "sq": sq_a,
                "ck": ck_a,
                "sk": sk_a,
            }
        )
    return in_maps


_NC_CACHE = {}


def get_graph():
    if "nc" not in _NC_CACHE:
        _NC_CACHE["nc"] = build_graph()
    return _NC_CACHE["nc"]


def run(inputs, trace=False, **kw):
    nc = get_graph()
    in_maps = prep_inputs(**inputs)
    res = run_bass_kernel_spmd(nc, in_maps, core_ids=list(range(NCORES)), trace=trace, **kw)
    out = np.empty((B, S, H), np.float32)
    for c in range(NCORES):
        b, j = c // GROUP, c % GROUP
        out[b, j * SLOC : (j + 1) * SLOC, :] = res.results[c]["out"]
    return out, res


def kernel(**inputs):
    out, _ = run(inputs, trace=False)
    return out


if __name__ == "__main__":
    print("building graph...")
    nc = get_graph()
    print("graph built and compiled")


# revision 24
# speedup vs baseline: 1.0092x; 1.0092x over previous
"""Distributed Trainium2 Bass kernel for nn_Attention_13125420057022.

Multi-head attention (B=2, S=2048, H=768, 12 heads, head_dim=64) with
interleaved RoPE, run SPMD on 8 NeuronCores.

Sharding: core c handles batch b=c//4 and query rows [512*(c%4), 512*(c%4+1)).
Data-parallel with replicated K/V: every core computes K^T and V for the FULL
sequence of its batch (the 4x redundant ~40us of warm TensorE time is cheaper
than a 4-rank ring AllGather, which measures ~90us serialized plus contended
read-back), plus Q for its own 512 rows. Attention and the output projection
then produce a disjoint slice of the output rows — no collectives at all.

Compute is bf16 with f32 PSUM accumulation. Scores are computed transposed
(S^T[k,q] = sum_d K^T[d,k] Q^T[d,q]) so the exp() output feeds the A.V matmul
directly with no on-chip transposes; two key blocks share one [128,1024] PSUM
tile so each ACT exp covers 1024 columns and TensorE runs long dependency-free
matmul waves (keeps the HAM clock at 2.4GHz). Softmax row-sums come from a
ones column appended to V; no max-subtraction is needed because the logits are
bounded. The per-query 1/sum broadcast runs on GpSimd (partition_broadcast);
PSUM evictions run on DVE so ACT does nothing but exp. RoPE uses a host-side
de-interleave permutation of the Q/K weight rows to turn interleaved rotation
into rotate-half form (contiguous 32-row block swaps via SB->SB DMA).
"""

import math
import sys
from contextlib import ExitStack

import numpy as np
import ml_dtypes

sys.path.insert(0, "/opt/trn_rl_repo")

import concourse.bass as bass  # noqa: E402
import concourse.mybir as mybir  # noqa: E402
import concourse.tile as tile  # noqa: E402
from concourse import bacc  # noqa: E402
from concourse.bass_utils import run_bass_kernel_spmd  # noqa: E402

BF16 = ml_dtypes.bfloat16
F32 = mybir.dt.float32
BF = mybir.dt.bfloat16

B, S, H = 2, 2048, 768
NH, HD = 12, 64
THETA = 10000.0
NCORES = 8
GROUP = 4  # cores per batch
SLOC = S // GROUP  # 512 query rows per core
NKB = S // 128  # 16 key blocks
NSC = S // 512  # 4 sequence chunks of 512

EXP = mybir.ActivationFunctionType.Exp


def build_graph():
    nc = bacc.Bacc(
        "TRN2",
        target_bir_lowering=False,
        debug=False,
        num_devices=NCORES,
    )

    # External inputs (per-core shards, host-prepped)
    xT = nc.dram_tensor("xT", [H, S], BF, kind="ExternalInput")  # full batch
    xTq = nc.dram_tensor("xTq", [H, SLOC], BF, kind="ExternalInput")  # own rows
    wt = nc.dram_tensor("wt", [H, 3 * H], BF, kind="ExternalInput")
    qkvb_qk = nc.dram_tensor("qkvb_qk", [128, 12], F32, kind="ExternalInput")
    qkvb_v = nc.dram_tensor("qkvb_v", [1, H], BF, kind="ExternalInput")
    projt = nc.dram_tensor("projt", [H, H], BF, kind="ExternalInput")
    projb = nc.dram_tensor("projb", [1, H], BF, kind="ExternalInput")
    cq = nc.dram_tensor("cq", [128, SLOC], BF, kind="ExternalInput")
    sq = nc.dram_tensor("sq", [128, SLOC], BF, kind="ExternalInput")
    ck = nc.dram_tensor("ck", [128, S], BF, kind="ExternalInput")
    sk = nc.dram_tensor("sk", [128, S], BF, kind="ExternalInput")
    out_ext = nc.dram_tensor("out", [SLOC, H], F32, kind="ExternalOutput")

    with tile.TileContext(nc) as tc, ExitStack() as ctx:
        singles = ctx.enter_context(tc.tile_pool(name="singles", bufs=1))
        kraw_p = ctx.enter_context(tc.tile_pool(name="kraw", bufs=2))
        kswp_p = ctx.enter_context(tc.tile_pool(name="kswp", bufs=2))
        ktmp_p = ctx.enter_context(tc.tile_pool(name="ktmp", bufs=1))
        qraw_p = ctx.enter_context(tc.tile_pool(name="qraw", bufs=2))
        qswp_p = ctx.enter_context(tc.tile_pool(name="qswp", bufs=2))
        qtmp_p = ctx.enter_context(tc.tile_pool(name="qtmp", bufs=1))
        v_pool = ctx.enter_context(tc.tile_pool(name="v_pool", bufs=1))
        at_pool = ctx.enter_context(tc.tile_pool(name="at", bufs=6))
        small_p = ctx.enter_context(tc.tile_pool(name="small", bufs=2))
        ctxn_p = ctx.enter_context(tc.tile_pool(name="ctxn", bufs=2))
        out_p = ctx.enter_context(tc.tile_pool(name="outp", bufs=2))

        # ---- SBUF tiles ----
        wt_sb = singles.tile([128, 6, 3 * H], BF)
        xT_sb = singles.tile([128, 6, S], BF)
        xTq_sb = singles.tile([128, 6, SLOC], BF)
        projt_sb = singles.tile([128, 6, H], BF)
        projb_sb = singles.tile([1, H], BF)
        qkvb_sb = singles.tile([128, 12], F32)
        qkvbv_sb = singles.tile([1, H], BF)
        cq_sb = singles.tile([128, SLOC], BF)
        sq_sb = singles.tile([128, SLOC], BF)
        ck_sb = singles.tile([128, S], BF)
        sk_sb = singles.tile([128, S], BF)
        ones_bf = singles.tile([1, 128], BF)
        qT_sb = singles.tile([128, 6, SLOC], BF)
        kT_sb = singles.tile([128, 6, S], BF)
        ctxT_sb = singles.tile([128, 6, SLOC], BF)

        wt_r = wt.ap().rearrange("(c p) n -> c p n", p=128)
        xT_r = xT.ap().rearrange("(c p) s -> c p s", p=128)
        xTq_r = xTq.ap().rearrange("(c p) s -> c p s", p=128)
        projt_r = projt.ap().rearrange("(c p) n -> c p n", p=128)
        # sync queue: x + K-columns of W interleaved so K block 0 starts
        # ASAP; the very first transfers are just the slices matmul 0 needs.
        nc.sync.dma_start(out=xT_sb[:, 0, 0:512], in_=xT_r[0][:, 0:512])
        nc.sync.dma_start(out=wt_sb[:, 0, 768:896], in_=wt_r[0][:, 768:896])
        nc.sync.dma_start(out=qkvb_sb[:], in_=qkvb_qk.ap())
        nc.sync.dma_start(out=xT_sb[:, 0, 512:1024], in_=xT_r[0][:, 512:1024])
        nc.sync.dma_start(out=wt_sb[:, 0, 896:1536], in_=wt_r[0][:, 896:1536])
        nc.sync.dma_start(out=xT_sb[:, 0, 1024:2048], in_=xT_r[0][:, 1024:2048])
        for c in range(1, 6):
            nc.sync.dma_start(out=xT_sb[:, c, 0:1024], in_=xT_r[c][:, 0:1024])
            nc.sync.dma_start(out=wt_sb[:, c, 768:1536], in_=wt_r[c][:, 768:1536])
            nc.sync.dma_start(out=xT_sb[:, c, 1024:2048], in_=xT_r[c][:, 1024:2048])
        # gpsimd (SWDGE) queue: V/Q weight columns + proj weights + own-x
        for c in range(6):
            nc.gpsimd.dma_start(out=wt_sb[:, c, 1536:2304], in_=wt_r[c][:, 1536:2304])
        for c in range(6):
            nc.gpsimd.dma_start(out=xTq_sb[:, c, :], in_=xTq_r[c])
        for c in range(6):
            nc.gpsimd.dma_start(out=wt_sb[:, c, 0:768], in_=wt_r[c][:, 0:768])
        for c in range(6):
            nc.gpsimd.dma_start(out=projt_sb[:, c, :], in_=projt_r[c])
        nc.gpsimd.dma_start(out=projb_sb[:], in_=projb.ap())
        # scalar queue: rope tables + v-bias
        nc.scalar.dma_start(out=ck_sb[:], in_=ck.ap())
        nc.scalar.dma_start(out=sk_sb[:], in_=sk.ap())
        nc.scalar.dma_start(out=cq_sb[:], in_=cq.ap())
        nc.scalar.dma_start(out=sq_sb[:], in_=sq.ap())
        nc.scalar.dma_start(out=qkvbv_sb[:], in_=qkvb_v.ap())
        nc.vector.memset(ones_bf[:], 1.0)

        def rope(dst, raw, swp, cos_sb, sin_sb, tmp_pool, width):
            """dst = raw*cos + swp*sin (rotate-half form, swp pre-swapped)."""
            t1 = tmp_pool.tile([128, width], BF, tag="t1")
            t2 = tmp_pool.tile([128, width], BF, tag="t2")
            nc.vector.tensor_mul(t1[:], raw[:], cos_sb[:])
            nc.vector.tensor_mul(t2[:], swp[:], sin_sb[:])
            nc.vector.tensor_add(dst, t1[:], t2[:])

        def swap_dmas(swp, raw, width):
            nc.scalar.dma_start(out=swp[0:32, 0:width], in_=raw[32:64, 0:width])
            nc.scalar.dma_start(out=swp[32:64, 0:width], in_=raw[0:32, 0:width])
            nc.scalar.dma_start(out=swp[64:96, 0:width], in_=raw[96:128, 0:width])
            nc.scalar.dma_start(out=swp[96:128, 0:width], in_=raw[64:96, 0:width])

        with tc.tile_pool(name="st_ps", bufs=3, space="PSUM") as st_psum:
            # ---- K^T for the full sequence (heads 2t, 2t+1 per block) ----
            for t in range(6):
                nb = t + 6  # K columns of wt
                kraw = kraw_p.tile([128, S], BF)
                for sp in range(2):  # two [128,1024] psum pairs -> 2048 cols
                    ps = st_psum.tile([128, 1024], F32, tag="st")
                    for half in range(2):
                        sc = 2 * sp + half
                        for c in range(6):
                            nc.tensor.matmul(
                                ps[:, half * 512 : (half + 1) * 512],
                                lhsT=wt_sb[:, c, nb * 128 : (nb + 1) * 128],
                                rhs=xT_sb[:, c, sc * 512 : (sc + 1) * 512],
                                start=(c == 0),
                                stop=(c == 5),
                            )
                    nc.vector.tensor_scalar_add(
                        kraw[:, sp * 1024 : (sp + 1) * 1024],
                        ps[:],
                        qkvb_sb[:, nb : nb + 1],
                    )
                kswp = kswp_p.tile([128, S], BF)
                swap_dmas(kswp, kraw, S)
                rope(kT_sb[:, t, :], kraw, kswp, ck_sb, sk_sb, ktmp_p, S)

            # ---- V for the full sequence (natural layout, ones column) ----
            v_tiles = []
            with tc.tile_pool(name="v_ps", bufs=1, space="PSUM") as v_psum:
                for i in range(NKB):
                    vps = v_psum.tile([128, H], F32)
                    for c in range(6):
                        lhsT = xT_sb[:, c, i * 128 : (i + 1) * 128]
                        nc.tensor.matmul(
                            vps[:, 0:512],
                            lhsT=lhsT,
                            rhs=wt_sb[:, c, 1536:2048],
                            start=(c == 0),
                            stop=False,
                        )
                        nc.tensor.matmul(
                            vps[:, 512:768],
                            lhsT=lhsT,
                            rhs=wt_sb[:, c, 2048:2304],
                            start=(c == 0),
                            stop=False,
                        )
                    # bias via ones-row (rank-1 update), also closes the groups
                    nc.tensor.matmul(
                        vps[:, 0:512],
                        lhsT=ones_bf[:, 0:128],
                        rhs=qkvbv_sb[:, 0:512],
                        start=False,
                        stop=True,
                    )
                    nc.tensor.matmul(
                        vps[:, 512:768],
                        lhsT=ones_bf[:, 0:128],
                        rhs=qkvbv_sb[:, 512:768],
                        start=False,
                        stop=True,
                    )
                    vt = v_pool.tile([128, NH * 65], BF, tag=f"v{i}")
                    vt3 = vt.rearrange("p (h c) -> p h c", h=NH)
                    nc.vector.tensor_copy(
                        vt3[:, :, 0:64], vps.rearrange("p (h d) -> p h d", h=NH)
                    )
                    nc.vector.memset(vt3[:, :, 64:65], 1.0)
                    v_tiles.append(vt)

            # ---- Q^T for own rows ----
            for t in range(6):
                ps = st_psum.tile([128, 1024], F32, tag="st")
                for c in range(6):
                    nc.tensor.matmul(
                        ps[:, 0:512],
                        lhsT=wt_sb[:, c, t * 128 : (t + 1) * 128],
                        rhs=xTq_sb[:, c, :],
                        start=(c == 0),
                        stop=(c == 5),
                    )
                qraw = qraw_p.tile([128, SLOC], BF)
                nc.vector.tensor_scalar_add(
                    qraw[:], ps[:, 0:512], qkvb_sb[:, t : t + 1]
                )
                qswp = qswp_p.tile([128, SLOC], BF)
                swap_dmas(qswp, qraw, SLOC)
                rope(qT_sb[:, t, :], qraw, qswp, cq_sb, sq_sb, qtmp_p, SLOC)

            # ---- attention: per head, paired S^T -> exp -> A^T.V ----
            with tc.tile_pool(name="ctx_ps", bufs=2, space="PSUM") as ctx_psum:

                def head_mm(h):
                    t, r0 = h // 2, (h % 2) * 64
                    ctxp = ctx_psum.tile([65, SLOC], F32, tag="ctx")
                    for kp in range(NKB // 2):
                        st = st_psum.tile([128, 1024], F32, tag="st")
                        at = at_pool.tile([128, 1024], BF)
                        for half in range(2):
                            kb = 2 * kp + half
                            nc.tensor.matmul(
                                st[:, half * 512 : (half + 1) * 512],
                                lhsT=kT_sb[r0 : r0 + 64, t, kb * 128 : (kb + 1) * 128],
                                rhs=qT_sb[r0 : r0 + 64, t, :],
                                start=True,
                                stop=True,
                            )
                        nc.scalar.activation(out=at[:], in_=st[:], func=EXP)
                        for half in range(2):
                            kb = 2 * kp + half
                            nc.tensor.matmul(
                                ctxp[:],
                                lhsT=v_tiles[kb][:, h * 65 : (h + 1) * 65],
                                rhs=at[:, half * 512 : (half + 1) * 512],
                                start=(kb == 0),
                                stop=(kb == NKB - 1),
                            )
                    return ctxp

                def head_epilogue(h, ctxp):
                    t, r0 = h // 2, (h % 2) * 64
                    sums64 = small_p.tile([128, SLOC], F32, tag="sums64")
                    nc.vector.tensor_copy(sums64[64:65, :], ctxp[64:65, :])
                    sums = small_p.tile([1, SLOC], F32, tag="sums")
                    nc.scalar.dma_start(out=sums[:], in_=sums64[64:65, :])
                    rec = small_p.tile([1, SLOC], F32, tag="rec")
                    nc.vector.reciprocal_approx_fast(out=rec[:], in_=sums[:])
                    bc_sb = small_p.tile([64, SLOC], F32, tag="bc_sb")
                    nc.gpsimd.partition_broadcast(bc_sb[:], rec[:], channels=64)
                    if r0 == 0:
                        nc.vector.tensor_mul(
                            ctxT_sb[0:64, t, :], ctxp[0:64, :], bc_sb[:]
                        )
                    else:
                        cn = ctxn_p.tile([64, SLOC], BF)
                        nc.vector.tensor_mul(cn[:], ctxp[0:64, :], bc_sb[:])
                        nc.scalar.dma_start(out=ctxT_sb[64:128, t, :], in_=cn[:])

                prev = None
                for h in range(NH):
                    ctxp = head_mm(h)
                    if prev is not None:
                        head_epilogue(h - 1, prev)
                    prev = ctxp
                head_epilogue(NH - 1, prev)

        # ---- output projection: out[s,:] = ctx^T.T @ projt + projb ----
        with tc.tile_pool(name="o_ps", bufs=2, space="PSUM") as o_psum:
            for i in range(4):
                ops = o_psum.tile([128, H], F32)
                for c in range(6):
                    lhsT = ctxT_sb[:, c, i * 128 : (i + 1) * 128]
                    nc.tensor.matmul(
                        ops[:, 0:512],
                        lhsT=lhsT,
                        rhs=projt_sb[:, c, 0:512],
                        start=(c == 0),
                        stop=False,
                    )
                    nc.tensor.matmul(
                        ops[:, 512:768],
                        lhsT=lhsT,
                        rhs=projt_sb[:, c, 512:768],
                        start=(c == 0),
                        stop=False,
                    )
                nc.tensor.matmul(
                    ops[:, 0:512],
                    lhsT=ones_bf[:, 0:128],
                    rhs=projb_sb[:, 0:512],
                    start=False,
                    stop=True,
                )
                nc.tensor.matmul(
                    ops[:, 512:768],
                    lhsT=ones_bf[:, 0:128],
                    rhs=projb_sb[:, 512:768],
                    start=False,
                    stop=True,
                )
                osb = out_p.tile([128, H], F32)
                nc.vector.tensor_copy(osb[:], ops[:])
                nc.sync.dma_start(
                    out=out_ext.ap()[i * 128 : (i + 1) * 128, :], in_=osb[:]
                )

    nc.compile()
    return nc


_PERM = np.concatenate([np.arange(0, HD, 2), np.arange(1, HD, 2)])


def prep_inputs(x, qkv_w, qkv_b, proj_w, proj_b):
    """Shard + lay out the full inputs into per-core input maps."""
    x = np.asarray(x, np.float32)
    qkv_w = np.asarray(qkv_w, np.float32)
    qkv_b = np.asarray(qkv_b, np.float32)
    proj_w = np.asarray(proj_w, np.float32)
    proj_b = np.asarray(proj_b, np.float32)

    # de-interleave permutation of q/k head dims (rows of qkv_w)
    Wp = qkv_w.copy()
    bp = qkv_b.copy()
    for sec in range(2):
        for h in range(NH):
            base = sec * H + h * HD
            Wp[base : base + HD] = qkv_w[base + _PERM]
            bp[base : base + HD] = qkv_b[base + _PERM]
    wt = np.ascontiguousarray(Wp.T).astype(BF16)  # [768, 2304]
    qkvb_qk = np.ascontiguousarray(bp[: 2 * H].reshape(12, 128).T).astype(np.float32)
    qkvb_v = qkv_b[2 * H :].reshape(1, H).astype(BF16)
    projt = np.ascontiguousarray(proj_w.T).astype(BF16)
    projb = proj_b.reshape(1, H).astype(BF16)

    inv_freq = 1.0 / (THETA ** (np.arange(0, HD, 2, dtype=np.float32) / HD))
    angles = np.arange(S, dtype=np.float32)[None, :] * inv_freq[:, None]  # [32, S]
    cos_t, sin_t = np.cos(angles), np.sin(angles)
    qscale = 1.0 / math.sqrt(HD)

    ck_a = np.tile(cos_t, (4, 1)).astype(BF16)  # [128, S]
    sk_a = np.concatenate([-sin_t, sin_t, -sin_t, sin_t], axis=0).astype(BF16)
    xT_full = [np.ascontiguousarray(x[b].T).astype(BF16) for b in range(B)]

    in_maps = []
    for c in range(NCORES):
        b, j = c // GROUP, c % GROUP
        sl = slice(j * SLOC, (j + 1) * SLOC)
        cos_j, sin_j = cos_t[:, sl] * qscale, sin_t[:, sl] * qscale
        cq_a = np.tile(cos_j, (4, 1)).astype(BF16)
        sq_a = np.concatenate([-sin_j, sin_j, -sin_j, sin_j], axis=0).astype(BF16)
        in_maps.append(
            {
                "xT": xT_full[b],
                "xTq": np.ascontiguousarray(xT_full[b][:, sl]),
                "wt": wt,
                "qkvb_qk": qkvb_qk,
                "qkvb_v": qkvb_v,
                "projt": projt,
                "projb": projb,
                "cq": cq_a,
                "sq": sq_a,
                "ck": ck_a,
                "sk": sk_a,
            }
        )
    return in_maps


_NC_CACHE = {}


def get_graph():
    if "nc" not in _NC_CACHE:
        _NC_CACHE["nc"] = build_graph()
    return _NC_CACHE["nc"]


def run(inputs, trace=False, **kw):
    nc = get_graph()
    in_maps = prep_inputs(**inputs)
    res = run_bass_kernel_spmd(nc, in_maps, core_ids=list(range(NCORES)), trace=trace, **kw)
    out = np.empty((B, S, H), np.float32)
    for c in range(NCORES):
        b, j = c // GROUP, c % GROUP
        out[b, j * SLOC : (j + 1) * SLOC, :] = res.results[c]["out"]
    return out, res


def kernel(**inputs):
    out, _ = run(inputs, trace=False)
    return out


if __name__ == "__main__":
    print("building graph...")
    nc = get_graph()
    print("graph built and compiled")
